# revision 48
# baseline (speedup 1.0000x reference)
"""Trainium2 Bass kernel for nn_CNNMnist_Sketch (sketched CNN forward pass).

Data-parallel over 8 NeuronCores: batch 4096 -> 512 per core.
Per-core pipeline (all shapes hardcoded):
  conv1 5x5 (1->32ch) + maxpool2 + relu   -> h1  [32ch, 12x12]
  conv2 5x5 (32->64ch) + maxpool2 + relu  -> h2  [64ch, 4x4] -> flat 1024
  fc1 1024->512 + relu, fc2 512->10, log_softmax

Layout/scheduling notes:
  - conv1: input replicated to 100 SBUF partitions (4 batch-chunks x 25 taps),
    each partition pre-shifted by its tap offset; a single block-diagonal
    [100,128] lhsT computes 4 chunks x 32 channels in one matmul stream.
  - conv2: pooled h1 bounced through DRAM and read back twice: h1r holds 4
    kw-shifted copies (shift 0..3), h1r2 holds 4 kh-shifted copies (shift
    0,12,24,36).  25 taps then contract in 7 passes (6 full-K + 1 K=32)
    instead of 10.  conv2 lhsT output channels are duplicated to M=128 so
    the PE activity monitor keeps the array at full clock (M=64 matmuls
    never unthrottle HAM from 1.2 to 2.4 GHz).
  - software pipelining: tensor queue order is conv1(k+1); conv2(k), so the
    relu/bounce/read chain for block k+1 hides under conv2(k)'s matmuls.
  - h2 lives in DRAM (feature-major) so fc1's k-chunk reads are contiguous.
"""

import numpy as np
import ml_dtypes

import concourse.bass as bass
import concourse.bacc as bacc
import concourse.tile as tile
from concourse import mybir
from concourse.bass_utils import run_bass_kernel_spmd

F32 = mybir.dt.float32
BF16 = mybir.dt.bfloat16
RELU = mybir.ActivationFunctionType.Relu
EXP = mybir.ActivationFunctionType.Exp
LN = mybir.ActivationFunctionType.Ln
MAXOP = mybir.AluOpType.max
SUBOP = mybir.AluOpType.subtract
ADDOP = mybir.AluOpType.add
AXY = mybir.AxisListType.XY
AX = mybir.AxisListType.X

NCORES = 8
BPC = 4096 // NCORES          # samples per core
BLK = 64                      # samples per block
NBLK = BPC // BLK
CS = BLK // 4                 # samples per conv1 chunk (4 chunks / block)
CHUNKF = CS * 784             # x elements per chunk
XBLK = BLK * 784              # x elements per block
H1F = CS * 144                # h1 elements per chunk (per channel)
XPAD = 128                    # DRAM pad so shifted reads never go OOB
HEAT = 40                     # warmup matmuls to unthrottle the PE clock

_CACHE = {}


def _build():
    nc = bacc.Bacc(target_bir_lowering=False, debug=False, num_devices=NCORES)

    xt = nc.dram_tensor("x", [BPC * 784 + XPAD], BF16, kind="ExternalInput").ap()
    wc1t = nc.dram_tensor("wc1bd", [128, 128], BF16, kind="ExternalInput").ap()
    w2at = nc.dram_tensor("w2all", [128, 10 * 128], BF16, kind="ExternalInput").ap()
    w3t = nc.dram_tensor("w3sb", [128, 4096], BF16, kind="ExternalInput").ap()
    fc2t = nc.dram_tensor("fc2sb", [128, 40], F32, kind="ExternalInput").ap()
    b1t = nc.dram_tensor("b1r", [128, 1], F32, kind="ExternalInput").ap()
    b2t = nc.dram_tensor("b2", [64, 1], F32, kind="ExternalInput").ap()
    b3t = nc.dram_tensor("b3sb", [128, 4], F32, kind="ExternalInput").ap()
    fbt = nc.dram_tensor("fc2b", [1, 10], F32, kind="ExternalInput").ap()
    ot = nc.dram_tensor("out", [BPC, 10], F32, kind="ExternalOutput").ap()

    from contextlib import ExitStack

    with tile.TileContext(nc, num_cores=NCORES) as tc, ExitStack() as es:
        W = es.enter_context(tc.tile_pool(name="weights", bufs=1))
        S = es.enter_context(tc.tile_pool(name="work", bufs=2))
        P = es.enter_context(tc.tile_pool(name="persist", bufs=1))
        PS = es.enter_context(tc.tile_pool(name="ps", bufs=8, space="PSUM"))
        DR = es.enter_context(tc.tile_pool(name="dram", bufs=2, space="DRAM"))
        DR2 = es.enter_context(tc.tile_pool(name="dram2", bufs=1, space="DRAM"))

        # ---- load weights (conv1's first; w3/fc2 deferred past xrep(0)) ----
        wc1 = W.tile([128, 128], BF16)
        nc.sync.dma_start(out=wc1[:], in_=wc1t)
        b1r = W.tile([128, 1], F32)
        nc.sync.dma_start(out=b1r[:], in_=b1t)
        w2a = W.tile([128, 1280], BF16)
        nc.scalar.dma_start(out=w2a[:], in_=w2at)
        b2 = W.tile([64, 1], F32)
        nc.scalar.dma_start(out=b2[:], in_=b2t)
        b3 = W.tile([128, 4], F32)
        nc.scalar.dma_start(out=b3[:], in_=b3t)
        ones1 = W.tile([1, 128], F32)
        nc.vector.memset(ones1[:], 1.0)
        heat = W.tile([128, 512], BF16)
        nc.vector.memset(heat[:], 1.0)

        h2d = DR2.tile([1024 * BPC], BF16, tag="h2d")

        def load_xrep(blk):
            """Shift-replication of block's x straight from DRAM:
            partition 25j+5kh+kw = chunk j shifted by 28*kh + kw.
            One issue per chunk j, spread over the three DMA queues."""
            xbase = blk * XBLK
            xrep = S.tile([128, CHUNKF + 8], BF16, tag="xrep")
            for j in range(4):
                src = bass.AP(
                    tensor=xt.tensor,
                    offset=xbase + j * CHUNKF,
                    ap=[[28, 5], [1, 5], [1, CHUNKF]],
                )
                nc.scalar.dma_start(
                    out=xrep[25 * j : 25 * j + 25, 0:CHUNKF], in_=src
                )
            # partitions 100-127 duplicate partitions 0-27 (half weight in
            # both rows) so conv1 contracts a full K=128 and the PE activity
            # monitor never sees a sparse-row phase
            src = bass.AP(
                tensor=xt.tensor,
                offset=xbase,
                ap=[[28, 5], [1, 5], [1, CHUNKF]],
            )
            nc.scalar.dma_start(out=xrep[100:125, 0:CHUNKF], in_=src)
            src = bass.AP(
                tensor=xt.tensor,
                offset=xbase + CHUNKF,
                ap=[[1, 3], [1, CHUNKF]],
            )
            nc.scalar.dma_start(out=xrep[125:128, 0:CHUNKF], in_=src)
            return xrep

        def conv1(xrep):
            """conv1 matmuls + pool; returns relu'd h1p [128=(j,co), H1F]."""
            h1p = S.tile([128, H1F + 8], BF16, tag="h1p")
            for s in range(CS):
                for h in range(2):
                    ps1 = PS.tile([128, 512], F32, tag="ps")
                    rhs = bass.AP(
                        tensor=xrep[:].tensor,
                        offset=xrep[:].offset + s * 784 + h * 336,
                        ap=[[CHUNKF + 8, 128], [28, 12], [1, 24]],
                    )
                    nc.tensor.matmul(
                        out=ps1[:, 0:288], lhsT=wc1[:], rhs=rhs,
                        start=True, stop=True,
                    )
                    pv = ps1[:, 0:288].rearrange(
                        "p (ph s1 pw s0) -> p ph pw s1 s0", ph=6, s1=2, pw=12, s0=2
                    )
                    ov = bass.AP(
                        tensor=h1p[:].tensor,
                        offset=h1p[:].offset + s * 144 + h * 72,
                        ap=[[H1F + 8, 128], [12, 6], [1, 12]],
                    )
                    nc.vector.tensor_reduce(out=ov, in_=pv, axis=AXY, op=MAXOP)
            nc.scalar.activation(
                h1p[:, 0:H1F], h1p[:, 0:H1F], RELU, bias=b1r[:]
            )
            return h1p

        def build_h1r(h1p):
            """DRAM bounce with big descriptors: h1p partitions are
            (ci,j)-interleaved (4*ci+j), so the write h1d[ci][j][f] is one
            partition-major contiguous DMA, and each kw-shift copy c is one
            32x18KB-descriptor read: h1r[(c,ci)][j,f] = h1d[ci*4H1F+j*H1F+f+c]."""
            h1d = DR.tile([128 * H1F + 16], BF16, tag="h1d")
            dst = bass.AP(
                tensor=h1d[:].tensor,
                offset=h1d[:].offset,
                ap=[[H1F, 128], [1, H1F]],
            )
            nc.sync.dma_start(out=dst, in_=h1p[:, 0:H1F])
            h1r = S.tile([128, 4 * H1F + 8], BF16, tag="h1r")
            for c in range(4):
                src = bass.AP(
                    tensor=h1d[:].tensor,
                    offset=h1d[:].offset + c,
                    ap=[[4 * H1F, 32], [1, 4 * H1F]],
                )
                nc.gpsimd.dma_start(
                    out=h1r[32 * c : 32 * c + 32, 0 : 4 * H1F], in_=src
                )
            return h1r

        def conv2(blk, h1r):
            """10-pass conv2 (pass-major over 4-group supergroups) + pool;
            writes relu'd pooled output to h2d DRAM (feature-major)."""
            h2s = S.tile([64, 1024], BF16, tag="h2s")
            for sg in range(2):            # j-pairs {0,1}, {2,3}
                groups = [(2 * sg + j, hh) for j in range(2) for hh in range(2)]
                banks = [
                    PS.tile([128, 512], F32, tag="ps", name=f"bank{gi}")
                    for gi in range(len(groups))
                ]
                # 10 full-K passes at offsets 12kh+e (e in {0,1}); each kw tap's
                # weight is split across the two passes that can reach it, so
                # every pass keeps all 128 PE rows active (HAM stays warm)
                passes = [
                    (128, 12 * kh + e,
                     w2a[:, 128 * (2 * kh + e) : 128 * (2 * kh + e) + 128])
                    for kh in range(5) for e in range(2)
                ]
                for p, (kk, off, lhsT) in enumerate(passes):
                    for gi, (j, hh) in enumerate(groups):
                        goff = j * H1F + hh * 8 * 144
                        rhs = bass.AP(
                            tensor=h1r[:].tensor,
                            offset=h1r[:].offset + goff + off,
                            ap=[[4 * H1F + 8, kk], [144, 8], [12, 8], [1, 8]],
                        )
                        nc.tensor.matmul(
                            out=banks[gi][:],
                            lhsT=lhsT,
                            rhs=rhs,
                            start=(p == 0),
                            stop=(p == 9),
                        )
                # pool conv2 8x8 -> 4x4 in two stages per group
                for gi, (j, hh) in enumerate(groups):
                    g = 2 * j + hh
                    st1 = S.tile([64, 256], F32, tag="st1")
                    iv = banks[gi][0:64, :].rearrange(
                        "p (soh pw s0) -> p soh pw s0", pw=4, s0=2
                    )
                    nc.vector.tensor_reduce(out=st1[:], in_=iv, axis=AX, op=MAXOP)
                    outv = bass.AP(
                        tensor=h2s[:].tensor,
                        offset=h2s[:].offset + g * 8,
                        ap=[[1024, 64], [256, 4], [64, 4], [1, 8]],
                    )
                    ia = bass.AP(
                        tensor=st1[:].tensor,
                        offset=st1[:].offset,
                        ap=[[256, 64], [8, 4], [1, 4], [32, 8]],
                    )
                    ib = bass.AP(
                        tensor=st1[:].tensor,
                        offset=st1[:].offset + 4,
                        ap=[[256, 64], [8, 4], [1, 4], [32, 8]],
                    )
                    nc.vector.tensor_tensor(out=outv, in0=ia, in1=ib, op=MAXOP)
            nc.scalar.activation(h2s[:], h2s[:], RELU, bias=b2[:])
            dst = bass.AP(
                tensor=h2d[:].tensor,
                offset=h2d[:].offset + blk * 64,
                ap=[[16 * BPC, 64], [BPC, 16], [1, 64]],
            )
            src = bass.AP(
                tensor=h2s[:].tensor,
                offset=h2s[:].offset,
                ap=[[1024, 64], [64, 16], [1, 64]],
            )
            nc.sync.dma_start(out=dst, in_=src)

        # fc1 runs in sample-halves so half 0 overlaps the conv block loop:
        # its k-chunk reads issue after block 3's h2d write, its matmuls
        # after conv2(4).  hr/h3 tiles persist across both halves.
        hr = [P.tile([128, BPC], BF16, tag=f"hr{kch}", name=f"hr{kch}")
              for kch in range(8)]
        h3 = [P.tile([128, BPC], F32, tag=f"h3{m}", name=f"h3{m}")
              for m in range(4)]

        def fc1_read(half):
            fo = half * 256
            for kch in range(8):
                src = bass.AP(
                    tensor=h2d[:].tensor,
                    offset=h2d[:].offset + kch * 128 * BPC + fo,
                    ap=[[BPC, 128], [1, 256]],
                )
                nc.sync.dma_start(out=hr[kch][:, fo : fo + 256], in_=src)

        def fc1_mm(half):
            fo = half * 256
            for m in range(4):
                psf = PS.tile([128, 256], F32, tag="ps", name=f"psf{m}")
                for kch in range(8):
                    nc.tensor.matmul(
                        out=psf[:],
                        lhsT=w3[:, (kch * 4 + m) * 128 : (kch * 4 + m) * 128 + 128],
                        rhs=hr[kch][:, fo : fo + 256],
                        start=(kch == 0),
                        stop=(kch == 7),
                    )
                nc.scalar.activation(
                    h3[m][:, fo : fo + 256], psf[:], RELU, bias=b3[:, m : m + 1]
                )

        # ---- prologue: prefetch x for block 0, warm up the PE ----
        xreps = {0: load_xrep(0)}
        for _ in range(HEAT):
            psh = PS.tile([128, 512], F32, tag="ps")
            nc.tensor.matmul(
                out=psh[:], lhsT=heat[:, 0:128], rhs=heat[:], start=True, stop=True
            )
        h1p0 = conv1(xreps.pop(0))
        reads = {0: build_h1r(h1p0)}
        xreps[1] = load_xrep(1)
        w3 = W.tile([128, 4096], BF16)
        nc.sync.dma_start(out=w3[:], in_=w3t)
        fc2 = W.tile([128, 40], F32)
        nc.sync.dma_start(out=fc2[:], in_=fc2t)
        fc2b = W.tile([1, 10], F32)
        nc.sync.dma_start(out=fc2b[:], in_=fbt)

        # ---- pipelined block loop: conv1(k+1) then conv2(k) ----
        for k in range(NBLK):
            if k + 2 < NBLK:
                xreps[k + 2] = load_xrep(k + 2)
            if k + 1 < NBLK:
                h1p = conv1(xreps.pop(k + 1))
                reads[k + 1] = build_h1r(h1p)
            conv2(k, reads.pop(k))
            if k == 3:
                fc1_read(0)
            if k == 4:
                fc1_mm(0)

        fc1_read(1)
        fc1_mm(1)

        # ---- fc2 + log_softmax, batch on partitions ----
        for bc in range(4):
            psl = PS.tile([128, 10], F32, tag="ps")
            for kch in range(4):
                nc.tensor.matmul(
                    out=psl[:],
                    lhsT=h3[kch][:, bc * 128 : bc * 128 + 128],
                    rhs=fc2[:, kch * 10 : kch * 10 + 10],
                    start=(kch == 0),
                    stop=False,
                )
            nc.tensor.matmul(
                out=psl[:],
                lhsT=ones1[:],
                rhs=fc2b[:],
                start=False,
                stop=True,
            )
            negm = S.tile([128, 1], F32, tag="negm")
            nc.vector.tensor_reduce(
                out=negm[:], in_=psl[:], axis=AX, op=MAXOP, negate=True
            )
            shifted = S.tile([128, 10], F32, tag="shifted")
            nc.vector.tensor_scalar(
                out=shifted[:], in0=psl[:], scalar1=negm[:], scalar2=None, op0=ADDOP
            )
            ex = S.tile([128, 10], F32, tag="ex")
            se = S.tile([128, 1], F32, tag="se")
            nc.scalar.activation(ex[:], shifted[:], EXP, accum_out=se[:])
            lse = S.tile([128, 1], F32, tag="lse")
            nc.scalar.activation(lse[:], se[:], LN)
            osb = S.tile([128, 10], F32, tag="osb")
            nc.vector.tensor_scalar(
                out=osb[:], in0=shifted[:], scalar1=lse[:], scalar2=None, op0=SUBOP
            )
            nc.sync.dma_start(out=ot[bc * 128 : bc * 128 + 128, :], in_=osb[:])

    nc.finalize()
    return nc


def _prep_weights(inputs):
    """Host-side: densify sketch weights and lay them out for the kernel."""
    h1, h2i, h3i = inputs["hash_idx1"], inputs["hash_idx2"], inputs["hash_idx3"]
    s1, s2, s3 = inputs["sgn1"], inputs["sgn2"], inputs["sgn3"]
    w1, w2, w3 = inputs["w1"], inputs["w2"], inputs["w3"]
    b1, b2, b3 = inputs["b1"], inputs["b2"], inputs["b3"]
    fc2w, fc2b = inputs["fc2_w"], inputs["fc2_b"]

    wc1 = (w1[:, h1] * s1[None, :]).astype(np.float32)            # (32, 25)
    wc2 = (w2[:, h2i] * s2[None, :]).astype(np.float32).reshape(64, 32, 5, 5)
    W3 = (w3[:, h3i] * s3[None, :]).astype(np.float32)            # (512, 1024)

    # block-diag conv1 lhsT; out partition = 4*co + j so the h1r
    # replication is a single partition-major walk per shift.  Rows
    # 100-127 duplicate rows 0-27 (xrep carries the same data there)
    # with the weight split in half, keeping all 128 PE rows active.
    wc1bd = np.zeros((128, 128), np.float32)
    for j in range(4):
        for co in range(32):
            wc1bd[25 * j : 25 * j + 25, 4 * co + j] = wc1[co, :]
    wc1bd[100:128, :] = 0.5 * wc1bd[0:28, :]
    wc1bd[0:28, :] *= 0.5

    # conv2 lhsT: 10 full-K passes at rhs offsets 12kh+e (e in {0,1}).
    # Pass (kh,e) row (c,ci) contracts tap (kh, kw=e+c); interior kw taps
    # are reachable from both passes and get half weight in each (exact in
    # bf16 -- halving only decrements the exponent).  Output channels are
    # duplicated to M=128 so the full PE array stays active.
    w2all = np.zeros((128, 10, 64), np.float32)
    for kh in range(5):
        for e in range(2):
            for c in range(4):
                kw = e + c
                scale = 1.0 if (kw == 0 or kw == 4) else 0.5
                w2all[32 * c : 32 * c + 32, 2 * kh + e, :] = (
                    scale * wc2[:, :, kh, kw].T
                )
    w2all = np.concatenate([w2all, w2all], axis=2).reshape(128, 1280)

    # fc1: lhsT chunk (k,m) = W3.T[128k:128k+128, 128m:128m+128]
    w3sb = np.zeros((128, 8, 4, 128), np.float32)
    W3T = np.ascontiguousarray(W3.T)  # (1024, 512)
    for k in range(8):
        for m in range(4):
            w3sb[:, k, m, :] = W3T[128 * k : 128 * k + 128, 128 * m : 128 * m + 128]
    w3sb = w3sb.reshape(128, 4096)

    fc2sb = np.zeros((128, 4, 10), np.float32)
    for k in range(4):
        fc2sb[:, k, :] = fc2w[:, 128 * k : 128 * k + 128].T
    fc2sb = fc2sb.reshape(128, 40)

    b1r = np.repeat(np.asarray(b1, np.float32), 4).reshape(128, 1)
    b3sb = np.asarray(b3, np.float32).reshape(4, 128).T.copy()

    bf = lambda a: np.asarray(a, dtype=ml_dtypes.bfloat16)
    f = lambda a: np.ascontiguousarray(a, dtype=np.float32)
    return {
        "wc1bd": bf(wc1bd),
        "w2all": bf(w2all),
        "w3sb": bf(w3sb),
        "fc2sb": f(fc2sb),
        "b1r": f(b1r),
        "b2": f(np.asarray(b2).reshape(64, 1)),
        "b3sb": f(b3sb),
        "fc2b": f(np.asarray(fc2b).reshape(1, 10)),
    }


def kernel(**inputs):
    out, _ = _run(inputs, trace=False)
    return out


def _run(inputs, trace=False):
    if "nc" not in _CACHE:
        _CACHE["nc"] = _build()
    nc = _CACHE["nc"]

    wmap = _prep_weights(inputs)
    x = np.asarray(inputs["x"], np.float32).reshape(4096, 784)

    in_maps = []
    for c in range(NCORES):
        xs = x[c * BPC : (c + 1) * BPC].reshape(-1)
        xs = np.concatenate([xs, np.zeros(XPAD, np.float32)])
        m = dict(wmap)
        m["x"] = np.asarray(xs, dtype=ml_dtypes.bfloat16)
        in_maps.append(m)

    res = run_bass_kernel_spmd(
        nc, in_maps, core_ids=list(range(NCORES)), trace=trace
    )
    out = np.concatenate([res.results[c]["out"] for c in range(NCORES)], axis=0)
    return out.astype(np.float32), res


# revision 53
# speedup vs baseline: 1.0518x; 1.0518x over previous
"""Trainium2 Bass kernel for nn_CNNMnist_Sketch (sketched CNN forward pass).

Data-parallel over 8 NeuronCores: batch 4096 -> 512 per core.
Per-core pipeline (all shapes hardcoded):
  conv1 5x5 (1->32ch) + maxpool2 + relu   -> h1  [32ch, 12x12]
  conv2 5x5 (32->64ch) + maxpool2 + relu  -> h2  [64ch, 4x4] -> flat 1024
  fc1 1024->512 + relu, fc2 512->10, log_softmax

Layout/scheduling notes:
  - conv1: input replicated to 100 SBUF partitions (4 batch-chunks x 25 taps),
    each partition pre-shifted by its tap offset; a single block-diagonal
    [100,128] lhsT computes 4 chunks x 32 channels in one matmul stream.
  - conv2: pooled h1 bounced through DRAM and read back twice: h1r holds 4
    kw-shifted copies (shift 0..3), h1r2 holds 4 kh-shifted copies (shift
    0,12,24,36).  25 taps then contract in 7 passes (6 full-K + 1 K=32)
    instead of 10.  conv2 lhsT output channels are duplicated to M=128 so
    the PE activity monitor keeps the array at full clock (M=64 matmuls
    never unthrottle HAM from 1.2 to 2.4 GHz).
  - software pipelining: tensor queue order is conv1(k+1); conv2(k), so the
    relu/bounce/read chain for block k+1 hides under conv2(k)'s matmuls.
  - h2 lives in DRAM (feature-major) so fc1's k-chunk reads are contiguous.
"""

import numpy as np
import ml_dtypes

import concourse.bass as bass
import concourse.bacc as bacc
import concourse.tile as tile
from concourse import mybir
from concourse.bass_utils import run_bass_kernel_spmd

F32 = mybir.dt.float32
BF16 = mybir.dt.bfloat16
RELU = mybir.ActivationFunctionType.Relu
EXP = mybir.ActivationFunctionType.Exp
LN = mybir.ActivationFunctionType.Ln
MAXOP = mybir.AluOpType.max
SUBOP = mybir.AluOpType.subtract
ADDOP = mybir.AluOpType.add
AXY = mybir.AxisListType.XY
AX = mybir.AxisListType.X

NCORES = 8
BPC = 4096 // NCORES          # samples per core
BLK = 64                      # samples per block
NBLK = BPC // BLK
CS = BLK // 4                 # samples per conv1 chunk (4 chunks / block)
CHUNKF = CS * 784             # x elements per chunk
XBLK = BLK * 784              # x elements per block
H1F = CS * 144                # h1 elements per chunk (per channel)
XPAD = 128                    # DRAM pad so shifted reads never go OOB
HEAT = 40                     # warmup matmuls to unthrottle the PE clock

_CACHE = {}


def _build():
    nc = bacc.Bacc(target_bir_lowering=False, debug=False, num_devices=NCORES)

    xt = nc.dram_tensor("x", [BPC * 784 + XPAD], BF16, kind="ExternalInput").ap()
    wc1t = nc.dram_tensor("wc1bd", [128, 128], BF16, kind="ExternalInput").ap()
    w2at = nc.dram_tensor("w2p", [128, 12 * 128], BF16, kind="ExternalInput").ap()
    w3t = nc.dram_tensor("w3sb", [128, 4096], BF16, kind="ExternalInput").ap()
    fc2t = nc.dram_tensor("fc2sb", [128, 40], F32, kind="ExternalInput").ap()
    b1t = nc.dram_tensor("b1r", [128, 1], F32, kind="ExternalInput").ap()
    b2t = nc.dram_tensor("b2", [64, 1], F32, kind="ExternalInput").ap()
    b3t = nc.dram_tensor("b3sb", [128, 4], F32, kind="ExternalInput").ap()
    fbt = nc.dram_tensor("fc2b", [1, 10], F32, kind="ExternalInput").ap()
    ot = nc.dram_tensor("out", [BPC, 10], F32, kind="ExternalOutput").ap()

    from contextlib import ExitStack

    with tile.TileContext(nc, num_cores=NCORES) as tc, ExitStack() as es:
        W = es.enter_context(tc.tile_pool(name="weights", bufs=1))
        S = es.enter_context(tc.tile_pool(name="work", bufs=2))
        P = es.enter_context(tc.tile_pool(name="persist", bufs=1))
        PS = es.enter_context(tc.tile_pool(name="ps", bufs=8, space="PSUM"))
        DR = es.enter_context(tc.tile_pool(name="dram", bufs=2, space="DRAM"))
        DR2 = es.enter_context(tc.tile_pool(name="dram2", bufs=1, space="DRAM"))

        # ---- load weights (conv1's first; w3/fc2 deferred past xrep(0)) ----
        wc1 = W.tile([128, 128], BF16)
        nc.sync.dma_start(out=wc1[:], in_=wc1t)
        b1r = W.tile([128, 1], F32)
        nc.sync.dma_start(out=b1r[:], in_=b1t)
        w2a = W.tile([128, 1536], BF16)
        nc.scalar.dma_start(out=w2a[:], in_=w2at)
        b2 = W.tile([64, 1], F32)
        nc.scalar.dma_start(out=b2[:], in_=b2t)
        b3 = W.tile([128, 4], F32)
        nc.scalar.dma_start(out=b3[:], in_=b3t)
        ones1 = W.tile([1, 128], F32)
        nc.vector.memset(ones1[:], 1.0)
        heat = W.tile([128, 512], BF16)
        nc.vector.memset(heat[:], 1.0)

        h2d = DR2.tile([1024 * BPC], BF16, tag="h2d")

        def load_xrep(blk):
            """Shift-replication of block's x straight from DRAM:
            partition 25j+5kh+kw = chunk j shifted by 28*kh + kw.
            One issue per chunk j, spread over the three DMA queues."""
            xbase = blk * XBLK
            xrep = S.tile([128, CHUNKF + 8], BF16, tag="xrep")
            for j in range(4):
                src = bass.AP(
                    tensor=xt.tensor,
                    offset=xbase + j * CHUNKF,
                    ap=[[28, 5], [1, 5], [1, CHUNKF]],
                )
                nc.scalar.dma_start(
                    out=xrep[25 * j : 25 * j + 25, 0:CHUNKF], in_=src
                )
            # partitions 100-127 duplicate partitions 0-27 (half weight in
            # both rows) so conv1 contracts a full K=128 and the PE activity
            # monitor never sees a sparse-row phase
            src = bass.AP(
                tensor=xt.tensor,
                offset=xbase,
                ap=[[28, 5], [1, 5], [1, CHUNKF]],
            )
            nc.scalar.dma_start(out=xrep[100:125, 0:CHUNKF], in_=src)
            src = bass.AP(
                tensor=xt.tensor,
                offset=xbase + CHUNKF,
                ap=[[1, 3], [1, CHUNKF]],
            )
            nc.scalar.dma_start(out=xrep[125:128, 0:CHUNKF], in_=src)
            return xrep

        def conv1(xrep):
            """conv1 matmuls + pool; returns relu'd h1p [128=(j,co), H1F]."""
            h1p = S.tile([128, H1F + 8], BF16, tag="h1p")
            for s in range(CS):
                for h in range(2):
                    ps1 = PS.tile([128, 512], F32, tag="ps")
                    rhs = bass.AP(
                        tensor=xrep[:].tensor,
                        offset=xrep[:].offset + s * 784 + h * 336,
                        ap=[[CHUNKF + 8, 128], [28, 12], [1, 24]],
                    )
                    nc.tensor.matmul(
                        out=ps1[:, 0:288], lhsT=wc1[:], rhs=rhs,
                        start=True, stop=True,
                    )
                    pv = ps1[:, 0:288].rearrange(
                        "p (ph s1 pw s0) -> p ph pw s1 s0", ph=6, s1=2, pw=12, s0=2
                    )
                    ov = bass.AP(
                        tensor=h1p[:].tensor,
                        offset=h1p[:].offset + s * 144 + h * 72,
                        ap=[[H1F + 8, 128], [12, 6], [1, 12]],
                    )
                    nc.vector.tensor_reduce(out=ov, in_=pv, axis=AXY, op=MAXOP)
            nc.scalar.activation(
                h1p[:, 0:H1F], h1p[:, 0:H1F], RELU, bias=b1r[:]
            )
            return h1p

        def build_h1r(h1p):
            """DRAM bounce with big descriptors: h1p partitions are
            (ci,j)-interleaved (4*ci+j), so the write h1d[ci][j][f] is one
            partition-major contiguous DMA, and each kw-shift copy c is one
            32x18KB-descriptor read: h1r[(c,ci)][j,f] = h1d[ci*4H1F+j*H1F+f+c]."""
            h1d = DR.tile([128 * H1F + 16], BF16, tag="h1d")
            dst = bass.AP(
                tensor=h1d[:].tensor,
                offset=h1d[:].offset,
                ap=[[H1F, 128], [1, H1F]],
            )
            nc.sync.dma_start(out=dst, in_=h1p[:, 0:H1F])
            h1r = S.tile([128, 4 * H1F + 8], BF16, tag="h1r")
            for c in range(4):
                src = bass.AP(
                    tensor=h1d[:].tensor,
                    offset=h1d[:].offset + c,
                    ap=[[4 * H1F, 32], [1, 4 * H1F]],
                )
                nc.gpsimd.dma_start(
                    out=h1r[32 * c : 32 * c + 32, 0 : 4 * H1F], in_=src
                )
            return h1r

        def conv2(blk, h1r):
            """Parity-packed conv2: out partitions = (parity, co) so each
            matmul computes two output rows at once (N=512 covers a full
            16-sample chunk).  12 full-K passes at offsets 12m+e; pass
            (m,e) row (c,ci) col (p,co) contracts tap (m-p, e+c).  Pool
            folds ow-pairs on parity 0, then maxes in the PSUM parity-1
            views directly (mixed SB+PSUM operands allow the cross-
            partition-base read)."""
            h2s = S.tile([64, 1024], BF16, tag="h2s")
            banks = [
                PS.tile([128, 512], F32, tag="ps", name=f"bank{j}")
                for j in range(4)
            ]
            for p in range(12):
                lhsT = w2a[:, 128 * p : 128 * p + 128]
                off = 12 * (p // 2) + (p % 2)
                for j in range(4):
                    rhs = bass.AP(
                        tensor=h1r[:].tensor,
                        offset=h1r[:].offset + j * H1F + off,
                        ap=[[4 * H1F + 8, 128], [144, 16], [24, 4], [1, 8]],
                    )
                    nc.tensor.matmul(
                        out=banks[j][:],
                        lhsT=lhsT,
                        rhs=rhs,
                        start=(p == 0),
                        stop=(p == 11),
                    )
            # pool 8x8 -> 4x4: fold ow-pairs of parity 0, then max with the
            # even/odd-ow strided views of parity 1 straight out of PSUM
            for j in range(4):
                stA = S.tile([64, 256], F32, tag="st1")
                ivA = banks[j][0:64, :].rearrange(
                    "p (s oh pw w2) -> p s oh pw w2", s=16, oh=4, pw=4, w2=2
                )
                nc.vector.tensor_reduce(out=stA[:], in_=ivA, axis=AX, op=MAXOP)
                pbase = banks[j][:].offset + 64 * 512
                even = bass.AP(
                    tensor=banks[j][:].tensor,
                    offset=pbase,
                    ap=[[512, 64], [32, 16], [8, 4], [2, 4]],
                )
                odd = bass.AP(
                    tensor=banks[j][:].tensor,
                    offset=pbase + 1,
                    ap=[[512, 64], [32, 16], [8, 4], [2, 4]],
                )
                t1 = S.tile([64, 256], F32, tag="t1")
                nc.vector.tensor_tensor(out=t1[:], in0=stA[:], in1=even, op=MAXOP)
                outv = bass.AP(
                    tensor=h2s[:].tensor,
                    offset=h2s[:].offset + 16 * j,
                    ap=[[1024, 64], [1, 16], [256, 4], [64, 4]],
                )
                nc.vector.tensor_tensor(out=outv, in0=t1[:], in1=odd, op=MAXOP)
            nc.scalar.activation(h2s[:], h2s[:], RELU, bias=b2[:])
            dst = bass.AP(
                tensor=h2d[:].tensor,
                offset=h2d[:].offset + blk * 64,
                ap=[[16 * BPC, 64], [BPC, 16], [1, 64]],
            )
            src = bass.AP(
                tensor=h2s[:].tensor,
                offset=h2s[:].offset,
                ap=[[1024, 64], [64, 16], [1, 64]],
            )
            nc.sync.dma_start(out=dst, in_=src)

        # fc1 runs in sample-halves so half 0 overlaps the conv block loop:
        # its k-chunk reads issue after block 3's h2d write, its matmuls
        # after conv2(4).  hr/h3 tiles persist across both halves.
        hr = [P.tile([128, BPC], BF16, tag=f"hr{kch}", name=f"hr{kch}")
              for kch in range(8)]
        h3 = [P.tile([128, BPC], F32, tag=f"h3{m}", name=f"h3{m}")
              for m in range(4)]

        def fc1_read(half):
            fo = half * 256
            for kch in range(8):
                src = bass.AP(
                    tensor=h2d[:].tensor,
                    offset=h2d[:].offset + kch * 128 * BPC + fo,
                    ap=[[BPC, 128], [1, 256]],
                )
                nc.sync.dma_start(out=hr[kch][:, fo : fo + 256], in_=src)

        def fc1_mm(half):
            fo = half * 256
            for m in range(4):
                psf = PS.tile([128, 256], F32, tag="ps", name=f"psf{m}")
                for kch in range(8):
                    nc.tensor.matmul(
                        out=psf[:],
                        lhsT=w3[:, (kch * 4 + m) * 128 : (kch * 4 + m) * 128 + 128],
                        rhs=hr[kch][:, fo : fo + 256],
                        start=(kch == 0),
                        stop=(kch == 7),
                    )
                nc.scalar.activation(
                    h3[m][:, fo : fo + 256], psf[:], RELU, bias=b3[:, m : m + 1]
                )

        # ---- prologue: prefetch x for block 0, warm up the PE ----
        xreps = {0: load_xrep(0)}
        for _ in range(HEAT):
            psh = PS.tile([128, 512], F32, tag="ps")
            nc.tensor.matmul(
                out=psh[:], lhsT=heat[:, 0:128], rhs=heat[:], start=True, stop=True
            )
        h1p0 = conv1(xreps.pop(0))
        reads = {0: build_h1r(h1p0)}
        xreps[1] = load_xrep(1)
        w3 = W.tile([128, 4096], BF16)
        nc.sync.dma_start(out=w3[:], in_=w3t)
        fc2 = W.tile([128, 40], F32)
        nc.sync.dma_start(out=fc2[:], in_=fc2t)
        fc2b = W.tile([1, 10], F32)
        nc.sync.dma_start(out=fc2b[:], in_=fbt)

        # ---- pipelined block loop: conv1(k+1) then conv2(k) ----
        for k in range(NBLK):
            if k + 2 < NBLK:
                xreps[k + 2] = load_xrep(k + 2)
            if k + 1 < NBLK:
                h1p = conv1(xreps.pop(k + 1))
                reads[k + 1] = build_h1r(h1p)
            conv2(k, reads.pop(k))
            if k == 3:
                fc1_read(0)
            if k == 4:
                fc1_mm(0)

        fc1_read(1)
        fc1_mm(1)

        # ---- fc2 + log_softmax, batch on partitions ----
        for bc in range(4):
            psl = PS.tile([128, 10], F32, tag="ps")
            for kch in range(4):
                nc.tensor.matmul(
                    out=psl[:],
                    lhsT=h3[kch][:, bc * 128 : bc * 128 + 128],
                    rhs=fc2[:, kch * 10 : kch * 10 + 10],
                    start=(kch == 0),
                    stop=False,
                )
            nc.tensor.matmul(
                out=psl[:],
                lhsT=ones1[:],
                rhs=fc2b[:],
                start=False,
                stop=True,
            )
            negm = S.tile([128, 1], F32, tag="negm")
            nc.vector.tensor_reduce(
                out=negm[:], in_=psl[:], axis=AX, op=MAXOP, negate=True
            )
            shifted = S.tile([128, 10], F32, tag="shifted")
            nc.vector.tensor_scalar(
                out=shifted[:], in0=psl[:], scalar1=negm[:], scalar2=None, op0=ADDOP
            )
            ex = S.tile([128, 10], F32, tag="ex")
            se = S.tile([128, 1], F32, tag="se")
            nc.scalar.activation(ex[:], shifted[:], EXP, accum_out=se[:])
            lse = S.tile([128, 1], F32, tag="lse")
            nc.scalar.activation(lse[:], se[:], LN)
            osb = S.tile([128, 10], F32, tag="osb")
            nc.vector.tensor_scalar(
                out=osb[:], in0=shifted[:], scalar1=lse[:], scalar2=None, op0=SUBOP
            )
            nc.sync.dma_start(out=ot[bc * 128 : bc * 128 + 128, :], in_=osb[:])

    nc.finalize()
    return nc


def _prep_weights(inputs):
    """Host-side: densify sketch weights and lay them out for the kernel."""
    h1, h2i, h3i = inputs["hash_idx1"], inputs["hash_idx2"], inputs["hash_idx3"]
    s1, s2, s3 = inputs["sgn1"], inputs["sgn2"], inputs["sgn3"]
    w1, w2, w3 = inputs["w1"], inputs["w2"], inputs["w3"]
    b1, b2, b3 = inputs["b1"], inputs["b2"], inputs["b3"]
    fc2w, fc2b = inputs["fc2_w"], inputs["fc2_b"]

    wc1 = (w1[:, h1] * s1[None, :]).astype(np.float32)            # (32, 25)
    wc2 = (w2[:, h2i] * s2[None, :]).astype(np.float32).reshape(64, 32, 5, 5)
    W3 = (w3[:, h3i] * s3[None, :]).astype(np.float32)            # (512, 1024)

    # block-diag conv1 lhsT; out partition = 4*co + j so the h1r
    # replication is a single partition-major walk per shift.  Rows
    # 100-127 duplicate rows 0-27 (xrep carries the same data there)
    # with the weight split in half, keeping all 128 PE rows active.
    wc1bd = np.zeros((128, 128), np.float32)
    for j in range(4):
        for co in range(32):
            wc1bd[25 * j : 25 * j + 25, 4 * co + j] = wc1[co, :]
    wc1bd[100:128, :] = 0.5 * wc1bd[0:28, :]
    wc1bd[0:28, :] *= 0.5

    # conv2 lhsT, parity-packed: out col = 64*parity + co computes output
    # row 2*ohh+parity.  12 full-K passes at rhs offsets 12m+e; pass (m,e)
    # row (c,ci) col (parity,co) contracts tap (kh=m-parity, kw=e+c).
    # Interior kw taps are reachable from both e passes and get half
    # weight in each (exact in bf16 -- halving only decrements the
    # exponent).
    w2p = np.zeros((128, 12, 128), np.float32)
    for m in range(6):
        for e in range(2):
            for c in range(4):
                kw = e + c
                if kw > 4:
                    continue
                scale = 1.0 if (kw == 0 or kw == 4) else 0.5
                for par in range(2):
                    kh = m - par
                    if not (0 <= kh <= 4):
                        continue
                    w2p[32 * c : 32 * c + 32, 2 * m + e,
                        64 * par : 64 * par + 64] = scale * wc2[:, :, kh, kw].T
    w2p = w2p.reshape(128, 1536)

    # fc1: lhsT chunk (k,m) = W3.T[128k:128k+128, 128m:128m+128]
    w3sb = np.zeros((128, 8, 4, 128), np.float32)
    W3T = np.ascontiguousarray(W3.T)  # (1024, 512)
    for k in range(8):
        for m in range(4):
            w3sb[:, k, m, :] = W3T[128 * k : 128 * k + 128, 128 * m : 128 * m + 128]
    w3sb = w3sb.reshape(128, 4096)

    fc2sb = np.zeros((128, 4, 10), np.float32)
    for k in range(4):
        fc2sb[:, k, :] = fc2w[:, 128 * k : 128 * k + 128].T
    fc2sb = fc2sb.reshape(128, 40)

    b1r = np.repeat(np.asarray(b1, np.float32), 4).reshape(128, 1)
    b3sb = np.asarray(b3, np.float32).reshape(4, 128).T.copy()

    bf = lambda a: np.asarray(a, dtype=ml_dtypes.bfloat16)
    f = lambda a: np.ascontiguousarray(a, dtype=np.float32)
    return {
        "wc1bd": bf(wc1bd),
        "w2p": bf(w2p),
        "w3sb": bf(w3sb),
        "fc2sb": f(fc2sb),
        "b1r": f(b1r),
        "b2": f(np.asarray(b2).reshape(64, 1)),
        "b3sb": f(b3sb),
        "fc2b": f(np.asarray(fc2b).reshape(1, 10)),
    }


def kernel(**inputs):
    out, _ = _run(inputs, trace=False)
    return out


def _run(inputs, trace=False):
    if "nc" not in _CACHE:
        _CACHE["nc"] = _build()
    nc = _CACHE["nc"]

    wmap = _prep_weights(inputs)
    x = np.asarray(inputs["x"], np.float32).reshape(4096, 784)

    in_maps = []
    for c in range(NCORES):
        xs = x[c * BPC : (c + 1) * BPC].reshape(-1)
        xs = np.concatenate([xs, np.zeros(XPAD, np.float32)])
        m = dict(wmap)
        m["x"] = np.asarray(xs, dtype=ml_dtypes.bfloat16)
        in_maps.append(m)

    res = run_bass_kernel_spmd(
        nc, in_maps, core_ids=list(range(NCORES)), trace=trace
    )
    out = np.concatenate([res.results[c]["out"] for c in range(NCORES)], axis=0)
    return out.astype(np.float32), res


# revision 62
# speedup vs baseline: 1.1038x; 1.0494x over previous
"""Trainium2 Bass kernel for nn_CNNMnist_Sketch (sketched CNN forward pass).

Data-parallel over 8 NeuronCores: batch 4096 -> 512 per core.
Per-core pipeline (all shapes hardcoded):
  conv1 5x5 (1->32ch) + maxpool2 + relu   -> h1  [32ch, 12x12]
  conv2 5x5 (32->64ch) + maxpool2 + relu  -> h2  [64ch, 4x4] -> flat 1024
  fc1 1024->512 + relu, fc2 512->10, log_softmax

Layout/scheduling notes:
  - conv1: input replicated to 100 SBUF partitions (4 batch-chunks x 25 taps),
    each partition pre-shifted by its tap offset; a single block-diagonal
    [100,128] lhsT computes 4 chunks x 32 channels in one matmul stream.
  - conv2: pooled h1 bounced through DRAM and read back twice: h1r holds 4
    kw-shifted copies (shift 0..3), h1r2 holds 4 kh-shifted copies (shift
    0,12,24,36).  25 taps then contract in 7 passes (6 full-K + 1 K=32)
    instead of 10.  conv2 lhsT output channels are duplicated to M=128 so
    the PE activity monitor keeps the array at full clock (M=64 matmuls
    never unthrottle HAM from 1.2 to 2.4 GHz).
  - software pipelining: tensor queue order is conv1(k+1); conv2(k), so the
    relu/bounce/read chain for block k+1 hides under conv2(k)'s matmuls.
  - h2 lives in DRAM (feature-major) so fc1's k-chunk reads are contiguous.
"""

import numpy as np
import ml_dtypes

import concourse.bass as bass
import concourse.bacc as bacc
import concourse.tile as tile
from concourse import mybir
from concourse.bass_utils import run_bass_kernel_spmd

F32 = mybir.dt.float32
BF16 = mybir.dt.bfloat16
RELU = mybir.ActivationFunctionType.Relu
EXP = mybir.ActivationFunctionType.Exp
LN = mybir.ActivationFunctionType.Ln
MAXOP = mybir.AluOpType.max
SUBOP = mybir.AluOpType.subtract
ADDOP = mybir.AluOpType.add
AXY = mybir.AxisListType.XY
AX = mybir.AxisListType.X

NCORES = 8
BPC = 4096 // NCORES          # samples per core
BLK = 64                      # samples per block
NBLK = BPC // BLK
CS = BLK // 4                 # samples per conv1 chunk (4 chunks / block)
CHUNKF = CS * 784             # x elements per chunk
XBLK = BLK * 784              # x elements per block
H1F = CS * 144                # h1 elements per chunk (per channel)
XPAD = 128                    # DRAM pad so shifted reads never go OOB
HEAT = 40                     # warmup matmuls to unthrottle the PE clock

_CACHE = {}


def _build():
    nc = bacc.Bacc(target_bir_lowering=False, debug=False, num_devices=NCORES)

    xt = nc.dram_tensor("x", [BPC * 784 + XPAD], BF16, kind="ExternalInput").ap()
    wc1t = nc.dram_tensor("wc1bd", [100, 128], BF16, kind="ExternalInput").ap()
    w2at = nc.dram_tensor("w2p", [128, 12 * 128], BF16, kind="ExternalInput").ap()
    w3t = nc.dram_tensor("w3sb", [128, 4096], BF16, kind="ExternalInput").ap()
    fc2t = nc.dram_tensor("fc2sb", [128, 40], F32, kind="ExternalInput").ap()
    b1t = nc.dram_tensor("b1r", [128, 1], F32, kind="ExternalInput").ap()
    b2t = nc.dram_tensor("b2", [64, 1], F32, kind="ExternalInput").ap()
    b3t = nc.dram_tensor("b3sb", [128, 4], F32, kind="ExternalInput").ap()
    fbt = nc.dram_tensor("fc2b", [1, 10], F32, kind="ExternalInput").ap()
    ot = nc.dram_tensor("out", [BPC, 10], F32, kind="ExternalOutput").ap()

    from contextlib import ExitStack

    with tile.TileContext(nc, num_cores=NCORES) as tc, ExitStack() as es:
        W = es.enter_context(tc.tile_pool(name="weights", bufs=1))
        S = es.enter_context(tc.tile_pool(name="work", bufs=2))
        P = es.enter_context(tc.tile_pool(name="persist", bufs=1))
        PS = es.enter_context(tc.tile_pool(name="ps", bufs=4, space="PSUM"))
        PS2 = es.enter_context(tc.tile_pool(name="ps2", bufs=4, space="PSUM"))
        DR = es.enter_context(tc.tile_pool(name="dram", bufs=2, space="DRAM"))
        DR2 = es.enter_context(tc.tile_pool(name="dram2", bufs=1, space="DRAM"))

        # ---- load weights (conv1's first; w3/fc2 deferred past xrep(0)) ----
        wc1 = W.tile([100, 128], BF16)
        nc.sync.dma_start(out=wc1[:], in_=wc1t)
        b1r = W.tile([128, 1], F32)
        nc.sync.dma_start(out=b1r[:], in_=b1t)
        w2a = W.tile([128, 1536], BF16)
        nc.scalar.dma_start(out=w2a[:], in_=w2at)
        b2 = W.tile([64, 1], F32)
        nc.scalar.dma_start(out=b2[:], in_=b2t)
        b3 = W.tile([128, 4], F32)
        nc.scalar.dma_start(out=b3[:], in_=b3t)
        ones1 = W.tile([1, 128], F32)
        nc.vector.memset(ones1[:], 1.0)
        heat = W.tile([128, 512], BF16)
        nc.vector.memset(heat[:], 1.0)

        h2d = DR2.tile([1024 * BPC], BF16, tag="h2d")

        def load_xrep(blk, spread=False):
            """Shift-replication of block's x straight from DRAM:
            partition 25j+5kh+kw = chunk j shifted by 28*kh + kw.
            Steady state keeps all issues on the scalar ring (ordering);
            the prologue spreads them across rings to fill faster."""
            xbase = blk * XBLK
            xrep = S.tile([100, CHUNKF + 8], BF16, tag="xrep")
            engs = (
                (nc.gpsimd, nc.sync, nc.scalar, nc.scalar)
                if spread
                else (nc.scalar,) * 4
            )
            for j in range(4):
                src = bass.AP(
                    tensor=xt.tensor,
                    offset=xbase + j * CHUNKF,
                    ap=[[28, 5], [1, 5], [1, CHUNKF]],
                )
                engs[j].dma_start(
                    out=xrep[25 * j : 25 * j + 25, 0:CHUNKF], in_=src
                )
            return xrep

        def conv1(xrep):
            """conv1 matmuls + pool; returns relu'd h1p [128=(j,co), H1F]."""
            h1p = S.tile([128, H1F + 8], BF16, tag="h1p")
            for s in range(CS):
                for h in range(2):
                    ps1 = PS.tile([128, 512], F32, tag="ps")
                    rhs = bass.AP(
                        tensor=xrep[:].tensor,
                        offset=xrep[:].offset + s * 784 + h * 336,
                        ap=[[CHUNKF + 8, 100], [28, 12], [1, 24]],
                    )
                    nc.tensor.matmul(
                        out=ps1[:, 0:288], lhsT=wc1[:], rhs=rhs,
                        start=True, stop=True,
                    )
                    pv = ps1[:, 0:288].rearrange(
                        "p (ph s1 pw s0) -> p ph pw s1 s0", ph=6, s1=2, pw=12, s0=2
                    )
                    ov = bass.AP(
                        tensor=h1p[:].tensor,
                        offset=h1p[:].offset + s * 144 + h * 72,
                        ap=[[H1F + 8, 128], [12, 6], [1, 12]],
                    )
                    nc.vector.tensor_reduce(out=ov, in_=pv, axis=AXY, op=MAXOP)
            nc.scalar.activation(
                h1p[:, 0:H1F], h1p[:, 0:H1F], RELU, bias=b1r[:]
            )
            return h1p

        def build_h1r(h1p):
            """DRAM bounce with big descriptors: h1p partitions are
            (ci,j)-interleaved (4*ci+j), so the write h1d[ci][j][f] is one
            partition-major contiguous DMA, and each kw-shift copy c is one
            32x18KB-descriptor read: h1r[(c,ci)][j,f] = h1d[ci*4H1F+j*H1F+f+c]."""
            h1d = DR.tile([128 * H1F + 16], BF16, tag="h1d")
            dst = bass.AP(
                tensor=h1d[:].tensor,
                offset=h1d[:].offset,
                ap=[[H1F, 128], [1, H1F]],
            )
            nc.sync.dma_start(out=dst, in_=h1p[:, 0:H1F])
            h1r = S.tile([128, 4 * H1F + 8], BF16, tag="h1r")
            for c in range(4):
                src = bass.AP(
                    tensor=h1d[:].tensor,
                    offset=h1d[:].offset + c,
                    ap=[[4 * H1F, 32], [1, 4 * H1F]],
                )
                nc.gpsimd.dma_start(
                    out=h1r[32 * c : 32 * c + 32, 0 : 4 * H1F], in_=src
                )
            return h1r

        def conv2(blk, h1r):
            """Parity-packed conv2: out partitions = (parity, co) so each
            matmul computes two output rows at once (N=512 covers a full
            16-sample chunk).  12 full-K passes at offsets 12m+e; pass
            (m,e) row (c,ci) col (p,co) contracts tap (m-p, e+c).  Pool
            folds ow-pairs on parity 0, then maxes in the PSUM parity-1
            views directly (mixed SB+PSUM operands allow the cross-
            partition-base read)."""
            h2s = S.tile([64, 1024], BF16, tag="h2s")
            banks = [
                PS2.tile([128, 512], F32, tag="c2", name=f"bank{j}")
                for j in range(4)
            ]
            for p in range(12):
                lhsT = w2a[:, 128 * p : 128 * p + 128]
                off = 12 * (p // 2) + (p % 2)
                for j in range(4):
                    rhs = bass.AP(
                        tensor=h1r[:].tensor,
                        offset=h1r[:].offset + j * H1F + off,
                        ap=[[4 * H1F + 8, 128], [144, 16], [24, 4], [1, 8]],
                    )
                    nc.tensor.matmul(
                        out=banks[j][:],
                        lhsT=lhsT,
                        rhs=rhs,
                        start=(p == 0),
                        stop=(p == 11),
                    )
            # pool 8x8 -> 4x4: fold ow-pairs of parity 0, then max with the
            # even/odd-ow strided views of parity 1 straight out of PSUM
            for j in range(4):
                stA = S.tile([64, 256], F32, tag="st1")
                ivA = banks[j][0:64, :].rearrange(
                    "p (s oh pw w2) -> p s oh pw w2", s=16, oh=4, pw=4, w2=2
                )
                nc.vector.tensor_reduce(out=stA[:], in_=ivA, axis=AX, op=MAXOP)
                pbase = banks[j][:].offset + 64 * 512
                even = bass.AP(
                    tensor=banks[j][:].tensor,
                    offset=pbase,
                    ap=[[512, 64], [32, 16], [8, 4], [2, 4]],
                )
                odd = bass.AP(
                    tensor=banks[j][:].tensor,
                    offset=pbase + 1,
                    ap=[[512, 64], [32, 16], [8, 4], [2, 4]],
                )
                t1 = S.tile([64, 256], F32, tag="t1")
                nc.vector.tensor_tensor(out=t1[:], in0=stA[:], in1=even, op=MAXOP)
                outv = bass.AP(
                    tensor=h2s[:].tensor,
                    offset=h2s[:].offset + 16 * j,
                    ap=[[1024, 64], [1, 16], [256, 4], [64, 4]],
                )
                nc.vector.tensor_tensor(out=outv, in0=t1[:], in1=odd, op=MAXOP)
            nc.scalar.activation(h2s[:], h2s[:], RELU, bias=b2[:])
            dst = bass.AP(
                tensor=h2d[:].tensor,
                offset=h2d[:].offset + blk * 64,
                ap=[[16 * BPC, 64], [BPC, 16], [1, 64]],
            )
            src = bass.AP(
                tensor=h2s[:].tensor,
                offset=h2s[:].offset,
                ap=[[1024, 64], [64, 16], [1, 64]],
            )
            nc.sync.dma_start(out=dst, in_=src)

        # fc1 runs in sample-halves so half 0 overlaps the conv block loop:
        # its k-chunk reads issue after block 3's h2d write, its matmuls
        # after conv2(4).  hr/h3 tiles persist across both halves.
        hr = [P.tile([128, BPC], BF16, tag=f"hr{kch}", name=f"hr{kch}")
              for kch in range(8)]
        h3 = [P.tile([128, BPC], F32, tag=f"h3{m}", name=f"h3{m}")
              for m in range(4)]

        def fc1_read(half):
            fo = half * 256
            for kch in range(8):
                src = bass.AP(
                    tensor=h2d[:].tensor,
                    offset=h2d[:].offset + kch * 128 * BPC + fo,
                    ap=[[BPC, 128], [1, 256]],
                )
                nc.sync.dma_start(out=hr[kch][:, fo : fo + 256], in_=src)

        def fc1_mm(half):
            fo = half * 256
            for m in range(4):
                psf = PS.tile([128, 256], F32, tag="ps", name=f"psf{m}")
                for kch in range(8):
                    nc.tensor.matmul(
                        out=psf[:],
                        lhsT=w3[:, (kch * 4 + m) * 128 : (kch * 4 + m) * 128 + 128],
                        rhs=hr[kch][:, fo : fo + 256],
                        start=(kch == 0),
                        stop=(kch == 7),
                    )
                nc.scalar.activation(
                    h3[m][:, fo : fo + 256], psf[:], RELU, bias=b3[:, m : m + 1]
                )

        # ---- prologue: prefetch x for block 0, warm up the PE ----
        xreps = {0: load_xrep(0, spread=True)}
        for _ in range(HEAT):
            psh = PS.tile([128, 512], F32, tag="ps")
            nc.tensor.matmul(
                out=psh[:], lhsT=heat[:, 0:128], rhs=heat[:], start=True, stop=True
            )
        h1p0 = conv1(xreps.pop(0))
        reads = {0: build_h1r(h1p0)}
        xreps[1] = load_xrep(1, spread=True)
        w3 = W.tile([128, 4096], BF16)
        nc.sync.dma_start(out=w3[:], in_=w3t)
        fc2 = W.tile([128, 40], F32)
        nc.sync.dma_start(out=fc2[:], in_=fc2t)
        fc2b = W.tile([1, 10], F32)
        nc.sync.dma_start(out=fc2b[:], in_=fbt)

        # ---- pipelined block loop: conv1(k+1) then conv2(k) ----
        for k in range(NBLK):
            if k + 2 < NBLK:
                xreps[k + 2] = load_xrep(k + 2)
            if k + 1 < NBLK:
                h1p = conv1(xreps.pop(k + 1))
                reads[k + 1] = build_h1r(h1p)
            conv2(k, reads.pop(k))
            if k == 3:
                fc1_read(0)
            if k == 4:
                fc1_mm(0)

        fc1_read(1)
        fc1_mm(1)

        # ---- fc2 + log_softmax, batch on partitions ----
        for bc in range(4):
            psl = PS.tile([128, 10], F32, tag="ps")
            for kch in range(4):
                nc.tensor.matmul(
                    out=psl[:],
                    lhsT=h3[kch][:, bc * 128 : bc * 128 + 128],
                    rhs=fc2[:, kch * 10 : kch * 10 + 10],
                    start=(kch == 0),
                    stop=False,
                )
            nc.tensor.matmul(
                out=psl[:],
                lhsT=ones1[:],
                rhs=fc2b[:],
                start=False,
                stop=True,
            )
            negm = S.tile([128, 1], F32, tag="negm")
            nc.vector.tensor_reduce(
                out=negm[:], in_=psl[:], axis=AX, op=MAXOP, negate=True
            )
            shifted = S.tile([128, 10], F32, tag="shifted")
            nc.vector.tensor_scalar(
                out=shifted[:], in0=psl[:], scalar1=negm[:], scalar2=None, op0=ADDOP
            )
            ex = S.tile([128, 10], F32, tag="ex")
            se = S.tile([128, 1], F32, tag="se")
            nc.scalar.activation(ex[:], shifted[:], EXP, accum_out=se[:])
            lse = S.tile([128, 1], F32, tag="lse")
            nc.scalar.activation(lse[:], se[:], LN)
            osb = S.tile([128, 10], F32, tag="osb")
            nc.vector.tensor_scalar(
                out=osb[:], in0=shifted[:], scalar1=lse[:], scalar2=None, op0=SUBOP
            )
            nc.sync.dma_start(out=ot[bc * 128 : bc * 128 + 128, :], in_=osb[:])

    nc.finalize()
    return nc


def _prep_weights(inputs):
    """Host-side: densify sketch weights and lay them out for the kernel."""
    h1, h2i, h3i = inputs["hash_idx1"], inputs["hash_idx2"], inputs["hash_idx3"]
    s1, s2, s3 = inputs["sgn1"], inputs["sgn2"], inputs["sgn3"]
    w1, w2, w3 = inputs["w1"], inputs["w2"], inputs["w3"]
    b1, b2, b3 = inputs["b1"], inputs["b2"], inputs["b3"]
    fc2w, fc2b = inputs["fc2_w"], inputs["fc2_b"]

    wc1 = (w1[:, h1] * s1[None, :]).astype(np.float32)            # (32, 25)
    wc2 = (w2[:, h2i] * s2[None, :]).astype(np.float32).reshape(64, 32, 5, 5)
    W3 = (w3[:, h3i] * s3[None, :]).astype(np.float32)            # (512, 1024)

    # block-diag conv1 lhsT; out partition = 4*co + j so the h1r
    # replication is a single partition-major walk per shift
    wc1bd = np.zeros((100, 128), np.float32)
    for j in range(4):
        for co in range(32):
            wc1bd[25 * j : 25 * j + 25, 4 * co + j] = wc1[co, :]

    # conv2 lhsT, parity-packed: out col = 64*parity + co computes output
    # row 2*ohh+parity.  12 full-K passes at rhs offsets 12m+e; pass (m,e)
    # row (c,ci) col (parity,co) contracts tap (kh=m-parity, kw=e+c).
    # Interior kw taps are reachable from both e passes and get half
    # weight in each (exact in bf16 -- halving only decrements the
    # exponent).
    w2p = np.zeros((128, 12, 128), np.float32)
    for m in range(6):
        for e in range(2):
            for c in range(4):
                kw = e + c
                if kw > 4:
                    continue
                scale = 1.0 if (kw == 0 or kw == 4) else 0.5
                for par in range(2):
                    kh = m - par
                    if not (0 <= kh <= 4):
                        continue
                    w2p[32 * c : 32 * c + 32, 2 * m + e,
                        64 * par : 64 * par + 64] = scale * wc2[:, :, kh, kw].T
    w2p = w2p.reshape(128, 1536)

    # fc1: lhsT chunk (k,m) = W3.T[128k:128k+128, 128m:128m+128]
    w3sb = np.zeros((128, 8, 4, 128), np.float32)
    W3T = np.ascontiguousarray(W3.T)  # (1024, 512)
    for k in range(8):
        for m in range(4):
            w3sb[:, k, m, :] = W3T[128 * k : 128 * k + 128, 128 * m : 128 * m + 128]
    w3sb = w3sb.reshape(128, 4096)

    fc2sb = np.zeros((128, 4, 10), np.float32)
    for k in range(4):
        fc2sb[:, k, :] = fc2w[:, 128 * k : 128 * k + 128].T
    fc2sb = fc2sb.reshape(128, 40)

    b1r = np.repeat(np.asarray(b1, np.float32), 4).reshape(128, 1)
    b3sb = np.asarray(b3, np.float32).reshape(4, 128).T.copy()

    bf = lambda a: np.asarray(a, dtype=ml_dtypes.bfloat16)
    f = lambda a: np.ascontiguousarray(a, dtype=np.float32)
    return {
        "wc1bd": bf(wc1bd),
        "w2p": bf(w2p),
        "w3sb": bf(w3sb),
        "fc2sb": f(fc2sb),
        "b1r": f(b1r),
        "b2": f(np.asarray(b2).reshape(64, 1)),
        "b3sb": f(b3sb),
        "fc2b": f(np.asarray(fc2b).reshape(1, 10)),
    }


def kernel(**inputs):
    out, _ = _run(inputs, trace=False)
    return out


def _run(inputs, trace=False):
    if "nc" not in _CACHE:
        _CACHE["nc"] = _build()
    nc = _CACHE["nc"]

    wmap = _prep_weights(inputs)
    x = np.asarray(inputs["x"], np.float32).reshape(4096, 784)

    in_maps = []
    for c in range(NCORES):
        xs = x[c * BPC : (c + 1) * BPC].reshape(-1)
        xs = np.concatenate([xs, np.zeros(XPAD, np.float32)])
        m = dict(wmap)
        m["x"] = np.asarray(xs, dtype=ml_dtypes.bfloat16)
        in_maps.append(m)

    res = run_bass_kernel_spmd(
        nc, in_maps, core_ids=list(range(NCORES)), trace=trace
    )
    out = np.concatenate([res.results[c]["out"] for c in range(NCORES)], axis=0)
    return out.astype(np.float32), res
